# revision 16
# baseline (speedup 1.0000x reference)
"""Two-layer single-head GAT (GATConv x2) on 8 trn2 NeuronCores.

Strategy: 1D node partition across 8 cores by destination node; edges live
with their destination owner, so edge-softmax and the scatter-aggregate stay
local. Weights replicated. Both layers share one graph plan / gather-index
tensor.

The bottleneck is SWDGE descriptor generation on gpsimd (~8ns/gathered row),
so the design minimizes gathered slots. dma_gather indices are int16, which
only reaches 32767 rows. Instead of splitting each destination's edges
across two gather windows (costs ~40% extra padded slots), each core's 49
destination tiles are split round-robin into 3 SUB-SHARDS; each sub-shard
gets its own COMPACT gather table holding just its distinct source nodes
(~27k rows < 32767), renumbered densely. Slot padding is then only the
per-tile max-degree padding (~2-3%).

Per layer, per core:
  Stage A (dense, PE, bf16): table rows T = h_src @ (W * a_src) written to
    the 3 concatenated sub-tables in DRAM (f32); host supplies hT with
    columns pre-arranged in sub-table order. ad = h_own @ (W @ a_dst).
    8 chunks per PSUM bank, batched 1024-row DMA writes.
  Stage B (sparse): destination tiles are degree-sorted, 128 dsts per tile
    (one per SBUF partition); tiles batched into gather GROUPS (sum K <= 96)
    within a sub-shard, ONE dma_gather per group. Per tile:
      as = rowsum(T_gathered)        (DVE reduce)
      s  = Lrelu(as + ad)            (Scalar activation, bias=ad)
      p  = Exp(s), den accumulated   (Scalar activation)
      rd = 1/den                     (DVE, batched per group)
      U  = sum_k p_k T_k             (DVE mult + transposed reduce)
      out = U * (1/a_src) * rd + b
  Padded slots point at each sub-table's row 0, filled with -1e30 => p == 0.
"""

import sys

sys.path.insert(0, "/opt/trn_rl_repo")

import numpy as np

N = 50000
E = 800000
IN = 128
OUT = 64
C = 8                       # cores
NSH = N // C                # 6250 dsts per core
NTILES = (NSH + 127) // 128  # 49
NSHP = NTILES * 128         # 6272 padded dsts per core
NEG_SLOPE = 0.2
NSUB = 4                    # sub-shards per core (tile t -> sub t % NSUB)
PAD_VAL = -1.0e30
GROUP_SLOT_BUDGET = 48      # max sum of K per gather group
LAST_SUB_BUDGET = 48        # smaller groups in the last sub-shard (short tail)


def _build_plan(edge_index):
    """Host-side graph preprocessing shared by both layers."""
    src = np.concatenate([np.asarray(edge_index[0], dtype=np.int64), np.arange(N)])
    dst = np.concatenate([np.asarray(edge_index[1], dtype=np.int64), np.arange(N)])

    core_of = dst // NSH
    orders = []
    pos_of = np.empty(N, dtype=np.int64)
    for c in range(C):
        d0 = c * NSH
        deg_c = np.bincount(dst[core_of == c] - d0, minlength=NSH)
        order = np.argsort(-deg_c, kind="stable")
        pos_of[d0 + order] = np.arange(NSH)
        orders.append(np.concatenate([order + d0, np.full(NSHP - NSH, -1, np.int64)]))

    epos = pos_of[dst]
    etile = epos // 128
    esub = etile % NSUB

    # per-tile K = max degree (over cores)
    K = np.zeros(NTILES, np.int64)
    for c in range(C):
        deg_p = np.bincount(epos[core_of == c], minlength=NSHP)
        K = np.maximum(K, deg_p.reshape(NTILES, 128).max(1))

    # per-(core, sub) distinct-source renumbering; local row 0 = PAD
    loc = np.zeros((C, NSUB, N), np.int32)
    n_cs = np.zeros((C, NSUB), np.int64)
    for c in range(C):
        for s in range(NSUB):
            nodes = np.unique(src[(core_of == c) & (esub == s)])
            n_cs[c, s] = len(nodes)
            loc[c, s, nodes] = 1 + np.arange(len(nodes), dtype=np.int32)
    subrows = int(n_cs.max()) + 1
    SUBROWS = ((subrows + 1023) // 1024) * 1024
    assert SUBROWS <= 32768, f"sub-table too big: {SUBROWS}"

    # per-edge slot assignment: rank within (core, pos)
    okey = np.lexsort((epos, core_of))
    sc, pc, srt = core_of[okey], epos[okey], src[okey]
    gid = sc * NSHP + pc
    first = np.r_[True, gid[1:] != gid[:-1]]
    idx_lin = np.arange(len(gid))
    start = np.maximum.accumulate(np.where(first, idx_lin, 0))
    rank = idx_lin - start
    assert (rank < K[(pc // 128)]).all()

    plan = dict(core=sc, pos=pc, src=srt, slot=rank, K=K)

    # gather groups: consecutive tiles of one sub-shard, sum K <= budget
    groups = []           # list of (sub, [tiles])
    for s in range(NSUB):
        budget = LAST_SUB_BUDGET if s == NSUB - 1 else GROUP_SLOT_BUDGET
        cur, acc = [], 0
        for t in range(s, NTILES, NSUB):
            k = int(K[t])
            if cur and acc + k > budget:
                groups.append((s, cur))
                cur, acc = [], 0
            cur.append(t)
            acc += k
        if cur:
            groups.append((s, cur))

    return orders, pos_of, plan, groups, loc, n_cs, SUBROWS


def _wrap_idx(arr):
    """[K,128] slot-major idx array -> [128, 8K] wrapped+replicated int16."""
    flat = arr.reshape(-1)                       # i = k*128 + p
    w = flat.reshape(-1, 16).T                   # [16, NI/16]
    return np.tile(w, (8, 1)).astype(np.int16)


def _build_idx_tensor(plan, groups, loc):
    """Per-core [128, IDXCOLS] int16 idx tensor (local sub-table rows)."""
    K = plan["K"]
    ginfo = []            # (sub, off, KG, [(t, tile_off, k), ...])
    off = 0
    for (s, tl) in groups:
        KG = int(sum(K[t] for t in tl))
        tiles = []
        toff = 0
        for t in tl:
            tiles.append((t, toff, int(K[t])))
            toff += int(K[t])
        ginfo.append((s, off, KG, tiles))
        off += 8 * KG
    idxcols = off

    core_a, pos_a, src_a, slot_a = plan["core"], plan["pos"], plan["src"], plan["slot"]
    out = np.zeros((C, 128, idxcols), np.int16)
    for c in range(C):
        m = core_a == c
        pos, srcn, slot = pos_a[m], src_a[m], slot_a[m]
        tile = pos // 128
        part = pos % 128
        for (s, goff, KG, tiles) in ginfo:
            arr = np.zeros((KG, 128), np.int64)             # pad -> row 0
            lc = loc[c, s]
            for (t, toff, k) in tiles:
                tm = tile == t
                arr[toff + slot[tm], part[tm]] = lc[srcn[tm]]
            out[c, :, goff:goff + 8 * KG] = _wrap_idx(arr)
    return out, ginfo, idxcols


def _build_launch(kdim, ginfo, idxcols, SUBROWS):
    """One SPMD launch: Stage A (sub-tables) + Stage B (gather groups)."""
    import concourse.bacc as bacc
    import concourse.mybir as mybir
    from concourse.tile import TileContext

    f32 = mybir.dt.float32
    bf16 = mybir.dt.bfloat16
    TROWS = NSUB * SUBROWS
    nchunk_sub = SUBROWS // 128
    SCH = 8                        # chunks per PSUM bank / super-chunk
    nsuper_sub = (nchunk_sub + SCH - 1) // SCH

    nc = bacc.Bacc(None, target_bir_lowering=False, debug=True)
    hT = nc.declare_dram_parameter("hT", [kdim, TROWS], bf16, isOutput=False)
    hoT = nc.declare_dram_parameter("hoT", [kdim, NSHP], bf16, isOutput=False)
    wse = nc.declare_dram_parameter("wse", [kdim, 65], bf16, isOutput=False)
    rb = nc.declare_dram_parameter("rb", [128, 128], f32, isOutput=False)
    idx = nc.declare_dram_parameter("idx", [128, idxcols], mybir.dt.int16,
                                    isOutput=False)
    outp = nc.declare_dram_parameter("outp", [NSHP, 64], f32, isOutput=True)
    tabl = nc.dram_tensor("tabl", [TROWS, 64], f32)

    with TileContext(nc) as tc:
        with (
            tc.tile_pool(name="const", bufs=1) as cpool,
            tc.tile_pool(name="xin", bufs=3) as xin,
            tc.tile_pool(name="stage", bufs=3) as stage,
            tc.tile_pool(name="psA", bufs=3, space="PSUM") as psA,
            tc.tile_pool(name="psB", bufs=2, space="PSUM") as psB,
            tc.tile_pool(name="tg", bufs=4) as tgp,
            tc.tile_pool(name="pt", bufs=2) as ptp,
            tc.tile_pool(name="sm", bufs=3) as sm,
        ):
            # idx slices per sub-shard, sub-0 first: the first gather prep
            # only waits on its own slice, not the whole 1.7MB index upload.
            sub_idx_range = {}
            for (s, goff, KG, tiles) in ginfo:
                lo, hi = sub_idx_range.get(s, (goff, goff + 8 * KG))
                sub_idx_range[s] = (min(lo, goff), max(hi, goff + 8 * KG))
            idx_sb = cpool.tile([128, idxcols], mybir.dt.int16)
            for s in sorted(sub_idx_range):
                lo, hi = sub_idx_range[s]
                nc.scalar.dma_start(out=idx_sb[:, lo:hi], in_=idx[:, lo:hi])
            wse_sb = cpool.tile([kdim, 65], bf16)
            nc.sync.dma_start(out=wse_sb[:, :], in_=wse[:, :])
            rb_sb = cpool.tile([128, 128], f32)
            nc.sync.dma_start(out=rb_sb[:, :], in_=rb[:, :])
            ho_sb = cpool.tile([kdim, NSHP], bf16)
            nc.sync.dma_start(out=ho_sb[:, :], in_=hoT[:, :])
            ad_sb = cpool.tile([128, NTILES], f32)
            padrow = cpool.tile([128, 64], f32)
            nc.vector.memset(padrow[:, :], PAD_VAL)

            # Stage A: per sub-table, 8 chunks per PSUM bank, batched
            # writes. The ad matmuls run after sub-0 so its gathers can
            # trigger as early as possible.
            for s in range(NSUB):
                if s == 1:
                    for t in range(NTILES):
                        ps2 = psB.tile([128, 1], f32, tag="ps2")
                        nc.tensor.matmul(ps2[:, :],
                                         ho_sb[:, 128 * t:128 * (t + 1)],
                                         wse_sb[:, 64:65], start=True,
                                         stop=True)
                        nc.scalar.copy(ad_sb[:, t:t + 1], ps2[:, :])
                base = s * SUBROWS
                for sci in range(nsuper_sub):
                    c0 = sci * SCH
                    nch = min(SCH, nchunk_sub - c0)
                    cols = 128 * nch
                    xt = xin.tile([kdim, 1024], bf16, tag="xt")
                    nc.sync.dma_start(
                        out=xt[:, 0:cols],
                        in_=hT[:, base + 128 * c0:base + 128 * c0 + cols])
                    ps = psA.tile([128, 512], f32, tag="ps")
                    for j in range(nch):
                        nc.tensor.matmul(ps[:, 64 * j:64 * (j + 1)],
                                         xt[:, 128 * j:128 * (j + 1)],
                                         wse_sb[:, 0:64], start=True, stop=True)
                    st = stage.tile([128, 512], f32, tag="st")
                    nc.vector.tensor_copy(st[:, 0:64 * nch], ps[:, 0:64 * nch])
                    dst = tabl[base + 128 * c0:base + 128 * c0 + cols, :] \
                        .rearrange("(c p) f -> p c f", p=128)
                    nc.scalar.dma_start(out=dst, in_=st[:, 0:64 * nch]
                                        .rearrange("p (c f) -> p c f", f=64))
                # pad row of this sub-table
                nc.sync.dma_start(out=tabl[base:base + 1, :], in_=padrow[0:1, :])

            # Stage B: gather groups. Desc-gen (prepare_only) has no table
            # dependency — it runs from t=0, overlapped with Stage A. All of
            # a sub-shard's preps are emitted before its ONE trigger so the
            # trigger's table-read wait never stalls later desc-gen.
            def compute_group(tg, dma_sem, tiles):
                ng = len(tiles)
                # tg consumers are all Vector ops; the prep's tick only covers
                # desc-gen, so gate Vector on the DMA-completion sem itself.
                nc.vector.wait_ge(dma_sem, 16)
                den_g = sm.tile([128, ng], f32, tag="den")
                p_list = []
                for i, (t, toff, k) in enumerate(tiles):
                    as_t = sm.tile([128, k], f32, tag=f"as{i}")
                    nc.vector.tensor_reduce(as_t[:, :],
                                            tg[:, toff:toff + k, :],
                                            mybir.AxisListType.X,
                                            mybir.AluOpType.add)
                    z_t = sm.tile([128, k], f32, tag=f"z{i}")
                    nc.vector.tensor_tensor(z_t[:, :], as_t[:, :],
                                            ad_sb[:, t:t + 1]
                                            .broadcast_to([128, k]),
                                            mybir.AluOpType.add)
                    s_t = sm.tile([128, k], f32, tag=f"s{i}")
                    nc.vector.scalar_tensor_tensor(s_t[:, :], z_t[:, :],
                                                   NEG_SLOPE, z_t[:, :],
                                                   mybir.AluOpType.mult,
                                                   mybir.AluOpType.max)
                    p_t = sm.tile([128, k], f32, tag=f"p{i}")
                    nc.scalar.activation(p_t[:, :], s_t[:, :],
                                         mybir.ActivationFunctionType.Exp,
                                         accum_out=den_g[:, i:i + 1])
                    p_list.append(p_t)
                rd_g = sm.tile([128, ng], f32, tag="rd")
                nc.vector.reciprocal(rd_g[:, :], den_g[:, :])
                for i, (t, toff, k) in enumerate(tiles):
                    p_t = p_list[i]
                    pt = ptp.tile([128, k, 64], f32, tag="pt")
                    p_b = p_t[:, :].unsqueeze(2).broadcast_to([128, k, 64])
                    nc.vector.tensor_tensor(pt[:, 0:k, :],
                                            tg[:, toff:toff + k, :], p_b,
                                            mybir.AluOpType.mult)
                    u = sm.tile([128, 64], f32, tag=f"u{i}")
                    nc.vector.tensor_reduce(u[:, :],
                                            pt[:, 0:k, :].transpose([0, 2, 1]),
                                            mybir.AxisListType.X,
                                            mybir.AluOpType.add)
                    o1 = sm.tile([128, 64], f32, tag=f"o1{i}")
                    nc.vector.scalar_tensor_tensor(o1[:, :], u[:, :],
                                                   rd_g[:, i:i + 1],
                                                   rb_sb[:, 0:64],
                                                   mybir.AluOpType.mult,
                                                   mybir.AluOpType.mult)
                    o2 = sm.tile([128, 64], f32, tag=f"o2{i}")
                    nc.vector.tensor_tensor(o2[:, :], o1[:, :],
                                            rb_sb[:, 64:128],
                                            mybir.AluOpType.add)
                    nc.sync.dma_start(out=outp[128 * t:128 * (t + 1), :],
                                      in_=o2[:, :])

            for gi, (s, goff, KG, tiles) in enumerate(ginfo):
                base = s * SUBROWS
                tg = tgp.tile([128, KG, 64], f32, tag="tg")
                dma_sem = nc.alloc_semaphore(f"swdge_g{gi}")
                nc.gpsimd.dma_gather(tg[:, :, :],
                                     tabl[base:base + SUBROWS, :],
                                     idx_sb[:, goff:goff + 8 * KG],
                                     128 * KG, 128 * KG, 64,
                                     single_packet=False,
                                     prepare_only=True, sem=dma_sem)
                nc.gpsimd.trigger_dma(count=None)
                compute_group(tg, dma_sem, tiles)

    nc.compile()
    return nc


LAST = {}


def _assemble_hT(featT_bf, loc, n_cs, SUBROWS, kdim):
    """Per-core hT [kdim, NSUB*SUBROWS] bf16 with sub-table column layout."""
    hts = []
    for c in range(C):
        ht = np.zeros((kdim, NSUB * SUBROWS), featT_bf.dtype)
        for s in range(NSUB):
            n = int(n_cs[c, s])
            nodes = np.nonzero(loc[c, s])[0]
            # loc values are 1..n in node order (np.unique sorted)
            ht[:, s * SUBROWS + 1:s * SUBROWS + 1 + n] = featT_bf[:, nodes]
        hts.append(ht)
    return hts


def kernel(x, edge_index, W1, a_src1, a_dst1, b1, W2, a_src2, a_dst2, b2):
    from concourse.bass_utils import run_bass_kernel_spmd
    import ml_dtypes

    bf = np.dtype(ml_dtypes.bfloat16)

    x = np.asarray(x, np.float32)
    edge_index = np.asarray(edge_index)
    W1 = np.asarray(W1, np.float32); a_src1 = np.asarray(a_src1, np.float32)
    a_dst1 = np.asarray(a_dst1, np.float32); b1 = np.asarray(b1, np.float32)
    W2 = np.asarray(W2, np.float32); a_src2 = np.asarray(a_src2, np.float32)
    a_dst2 = np.asarray(a_dst2, np.float32); b2 = np.asarray(b2, np.float32)

    orders, pos_of, plan, groups, loc, n_cs, SUBROWS = _build_plan(edge_index)
    idx, ginfo, idxcols = _build_idx_tensor(plan, groups, loc)

    nc1 = _build_launch(IN, ginfo, idxcols, SUBROWS)
    nc2 = _build_launch(OUT, ginfo, idxcols, SUBROWS)

    def guard(a):
        return np.where(a == 0, np.float32(1e-30), a)

    w1se = np.concatenate([W1 * a_src1[None, :], (W1 @ a_dst1)[:, None]], 1).astype(bf)
    w2se = np.concatenate([W2 * a_src2[None, :], (W2 @ a_dst2)[:, None]], 1).astype(bf)
    rb1 = np.concatenate([np.tile(1.0 / guard(a_src1), (128, 1)),
                          np.tile(b1, (128, 1))], 1).astype(np.float32)
    rb2 = np.concatenate([np.tile(1.0 / guard(a_src2), (128, 1)),
                          np.tile(b2, (128, 1))], 1).astype(np.float32)

    # layer 1 inputs
    xT_bf = np.ascontiguousarray(x.T).astype(bf)            # [IN, N]
    hts1 = _assemble_hT(xT_bf, loc, n_cs, SUBROWS, IN)
    in_maps1 = []
    for c in range(C):
        own = orders[c]
        hoT = np.zeros((IN, NSHP), bf)
        real = own >= 0
        hoT[:, real] = xT_bf[:, own[real]]
        in_maps1.append({"hT": hts1[c], "hoT": hoT, "wse": w1se, "rb": rb1,
                        "idx": idx[c]})

    res1 = run_bass_kernel_spmd(nc1, in_maps1, core_ids=list(range(C)))
    LAST["res1"] = res1

    # h2 per node from pi-order shards
    h2 = np.zeros((N, OUT), np.float32)
    for c in range(C):
        sh = np.asarray(res1.results[c]["outp"])
        own = orders[c]
        real = own >= 0
        h2[own[real]] = sh[real]
    h2T_bf = np.ascontiguousarray(h2.T).astype(bf)          # [64, N]

    hts2 = _assemble_hT(h2T_bf, loc, n_cs, SUBROWS, OUT)
    in_maps2 = []
    for c in range(C):
        own = orders[c]
        hoT2 = np.zeros((OUT, NSHP), bf)
        real = own >= 0
        hoT2[:, real] = h2T_bf[:, own[real]]
        in_maps2.append({"hT": hts2[c], "hoT": hoT2, "wse": w2se, "rb": rb2,
                        "idx": idx[c]})

    res2 = run_bass_kernel_spmd(nc2, in_maps2, core_ids=list(range(C)))
    LAST["res2"] = res2

    out = np.empty((N, OUT), np.float32)
    for c in range(C):
        sh = np.asarray(res2.results[c]["outp"])
        own = orders[c]
        real = own >= 0
        out[own[real]] = sh[real]
    return out


# revision 17
# speedup vs baseline: 1.1724x; 1.1724x over previous
"""Two-layer single-head GAT (GATConv x2) on 8 trn2 NeuronCores.

Strategy: 1D node partition across 8 cores by destination node; edges live
with their destination owner, so edge-softmax and the scatter-aggregate stay
local. Weights replicated. Both layers share one graph plan / gather-index
tensor.

The bottleneck is SWDGE descriptor generation on gpsimd (~8ns/gathered row),
so the design minimizes gathered slots. dma_gather indices are int16, which
only reaches 32767 rows. Instead of splitting each destination's edges
across two gather windows (costs ~40% extra padded slots), each core's 49
destination tiles are split round-robin into 3 SUB-SHARDS; each sub-shard
gets its own COMPACT gather table holding just its distinct source nodes
(~27k rows < 32767), renumbered densely. Slot padding is then only the
per-tile max-degree padding (~2-3%).

Per layer, per core:
  Stage A (dense, PE, bf16): table rows T = h_src @ (W * a_src) written to
    the 3 concatenated sub-tables in DRAM (f32); host supplies hT with
    columns pre-arranged in sub-table order. ad = h_own @ (W @ a_dst).
    8 chunks per PSUM bank, batched 1024-row DMA writes.
  Stage B (sparse): destination tiles are degree-sorted, 128 dsts per tile
    (one per SBUF partition); tiles batched into gather GROUPS (sum K <= 96)
    within a sub-shard, ONE dma_gather per group. Per tile:
      as = rowsum(T_gathered)        (DVE reduce)
      s  = Lrelu(as + ad)            (Scalar activation, bias=ad)
      p  = Exp(s), den accumulated   (Scalar activation)
      rd = 1/den                     (DVE, batched per group)
      U  = sum_k p_k T_k             (DVE mult + transposed reduce)
      out = U * (1/a_src) * rd + b
  Padded slots point at each sub-table's row 0, filled with -1e30 => p == 0.
"""

import sys

sys.path.insert(0, "/opt/trn_rl_repo")

import numpy as np

N = 50000
E = 800000
IN = 128
OUT = 64
C = 8                       # cores
NSH = N // C                # 6250 dsts per core
NTILES = (NSH + 127) // 128  # 49
NSHP = NTILES * 128         # 6272 padded dsts per core
NEG_SLOPE = 0.2
NSUB = 4                    # sub-shards per core (tile t -> sub t % NSUB)
PAD_VAL = -1.0e30
GROUP_SLOT_BUDGET = 96      # max sum of K per gather group
LAST_SUB_BUDGET = 56        # smaller groups in the last sub-shard (short tail)


def _build_plan(edge_index):
    """Host-side graph preprocessing shared by both layers."""
    src = np.concatenate([np.asarray(edge_index[0], dtype=np.int64), np.arange(N)])
    dst = np.concatenate([np.asarray(edge_index[1], dtype=np.int64), np.arange(N)])

    core_of = dst // NSH
    orders = []
    pos_of = np.empty(N, dtype=np.int64)
    for c in range(C):
        d0 = c * NSH
        deg_c = np.bincount(dst[core_of == c] - d0, minlength=NSH)
        order = np.argsort(-deg_c, kind="stable")
        pos_of[d0 + order] = np.arange(NSH)
        orders.append(np.concatenate([order + d0, np.full(NSHP - NSH, -1, np.int64)]))

    epos = pos_of[dst]
    etile = epos // 128
    esub = etile % NSUB

    # per-tile K = max degree (over cores)
    K = np.zeros(NTILES, np.int64)
    for c in range(C):
        deg_p = np.bincount(epos[core_of == c], minlength=NSHP)
        K = np.maximum(K, deg_p.reshape(NTILES, 128).max(1))

    # per-(core, sub) distinct-source renumbering; local row 0 = PAD
    loc = np.zeros((C, NSUB, N), np.int32)
    n_cs = np.zeros((C, NSUB), np.int64)
    for c in range(C):
        for s in range(NSUB):
            nodes = np.unique(src[(core_of == c) & (esub == s)])
            n_cs[c, s] = len(nodes)
            loc[c, s, nodes] = 1 + np.arange(len(nodes), dtype=np.int32)
    subrows = int(n_cs.max()) + 1
    SUBROWS = ((subrows + 1023) // 1024) * 1024
    assert SUBROWS <= 32768, f"sub-table too big: {SUBROWS}"

    # per-edge slot assignment: rank within (core, pos)
    okey = np.lexsort((epos, core_of))
    sc, pc, srt = core_of[okey], epos[okey], src[okey]
    gid = sc * NSHP + pc
    first = np.r_[True, gid[1:] != gid[:-1]]
    idx_lin = np.arange(len(gid))
    start = np.maximum.accumulate(np.where(first, idx_lin, 0))
    rank = idx_lin - start
    assert (rank < K[(pc // 128)]).all()

    plan = dict(core=sc, pos=pc, src=srt, slot=rank, K=K)

    # gather groups: consecutive tiles of one sub-shard, sum K <= budget
    groups = []           # list of (sub, [tiles])
    for s in range(NSUB):
        budget = LAST_SUB_BUDGET if s == NSUB - 1 else GROUP_SLOT_BUDGET
        cur, acc = [], 0
        for t in range(s, NTILES, NSUB):
            k = int(K[t])
            if cur and acc + k > budget:
                groups.append((s, cur))
                cur, acc = [], 0
            cur.append(t)
            acc += k
        if cur:
            groups.append((s, cur))

    return orders, pos_of, plan, groups, loc, n_cs, SUBROWS


def _wrap_idx(arr):
    """[K,128] slot-major idx array -> [128, 8K] wrapped+replicated int16."""
    flat = arr.reshape(-1)                       # i = k*128 + p
    w = flat.reshape(-1, 16).T                   # [16, NI/16]
    return np.tile(w, (8, 1)).astype(np.int16)


def _build_idx_tensor(plan, groups, loc):
    """Per-core [128, IDXCOLS] int16 idx tensor (local sub-table rows)."""
    K = plan["K"]
    ginfo = []            # (sub, off, KG, [(t, tile_off, k), ...])
    off = 0
    for (s, tl) in groups:
        KG = int(sum(K[t] for t in tl))
        tiles = []
        toff = 0
        for t in tl:
            tiles.append((t, toff, int(K[t])))
            toff += int(K[t])
        ginfo.append((s, off, KG, tiles))
        off += 8 * KG
    idxcols = off

    core_a, pos_a, src_a, slot_a = plan["core"], plan["pos"], plan["src"], plan["slot"]
    out = np.zeros((C, 128, idxcols), np.int16)
    for c in range(C):
        m = core_a == c
        pos, srcn, slot = pos_a[m], src_a[m], slot_a[m]
        tile = pos // 128
        part = pos % 128
        for (s, goff, KG, tiles) in ginfo:
            arr = np.zeros((KG, 128), np.int64)             # pad -> row 0
            lc = loc[c, s]
            for (t, toff, k) in tiles:
                tm = tile == t
                arr[toff + slot[tm], part[tm]] = lc[srcn[tm]]
            out[c, :, goff:goff + 8 * KG] = _wrap_idx(arr)
    return out, ginfo, idxcols


def _build_launch(kdim, ginfo, idxcols, SUBROWS):
    """One SPMD launch: Stage A (sub-tables) + Stage B (gather groups)."""
    import concourse.bacc as bacc
    import concourse.mybir as mybir
    from concourse.tile import TileContext

    f32 = mybir.dt.float32
    bf16 = mybir.dt.bfloat16
    TROWS = NSUB * SUBROWS
    nchunk_sub = SUBROWS // 128
    SCH = 8                        # chunks per PSUM bank / super-chunk
    nsuper_sub = (nchunk_sub + SCH - 1) // SCH

    nc = bacc.Bacc(None, target_bir_lowering=False, debug=True)
    hT = nc.declare_dram_parameter("hT", [kdim, TROWS], bf16, isOutput=False)
    hoT = nc.declare_dram_parameter("hoT", [kdim, NSHP], bf16, isOutput=False)
    wse = nc.declare_dram_parameter("wse", [kdim, 65], bf16, isOutput=False)
    rb = nc.declare_dram_parameter("rb", [128, 128], f32, isOutput=False)
    idx = nc.declare_dram_parameter("idx", [128, idxcols], mybir.dt.int16,
                                    isOutput=False)
    outp = nc.declare_dram_parameter("outp", [NSHP, 64], f32, isOutput=True)
    tabl = nc.dram_tensor("tabl", [TROWS, 64], f32)

    with TileContext(nc) as tc:
        with (
            tc.tile_pool(name="const", bufs=1) as cpool,
            tc.tile_pool(name="xin", bufs=3) as xin,
            tc.tile_pool(name="stage", bufs=3) as stage,
            tc.tile_pool(name="psA", bufs=3, space="PSUM") as psA,
            tc.tile_pool(name="psB", bufs=2, space="PSUM") as psB,
            tc.tile_pool(name="tg", bufs=4) as tgp,
            tc.tile_pool(name="pt", bufs=2) as ptp,
            tc.tile_pool(name="sm", bufs=3) as sm,
        ):
            # idx slices per sub-shard, sub-0 first: the first gather prep
            # only waits on its own slice, not the whole 1.7MB index upload.
            sub_idx_range = {}
            for (s, goff, KG, tiles) in ginfo:
                lo, hi = sub_idx_range.get(s, (goff, goff + 8 * KG))
                sub_idx_range[s] = (min(lo, goff), max(hi, goff + 8 * KG))
            idx_sb = cpool.tile([128, idxcols], mybir.dt.int16)
            for s in sorted(sub_idx_range):
                lo, hi = sub_idx_range[s]
                nc.scalar.dma_start(out=idx_sb[:, lo:hi], in_=idx[:, lo:hi])
            wse_sb = cpool.tile([kdim, 65], bf16)
            nc.sync.dma_start(out=wse_sb[:, :], in_=wse[:, :])
            rb_sb = cpool.tile([128, 128], f32)
            nc.sync.dma_start(out=rb_sb[:, :], in_=rb[:, :])
            ho_sb = cpool.tile([kdim, NSHP], bf16)
            nc.sync.dma_start(out=ho_sb[:, :], in_=hoT[:, :])
            ad_sb = cpool.tile([128, NTILES], f32)
            padrow = cpool.tile([128, 64], f32)
            nc.vector.memset(padrow[:, :], PAD_VAL)

            # Stage A: per sub-table, 8 chunks per PSUM bank, batched
            # writes. The ad matmuls run after sub-0 so its gathers can
            # trigger as early as possible.
            for s in range(NSUB):
                if s == 1:
                    for t in range(NTILES):
                        ps2 = psB.tile([128, 1], f32, tag="ps2")
                        nc.tensor.matmul(ps2[:, :],
                                         ho_sb[:, 128 * t:128 * (t + 1)],
                                         wse_sb[:, 64:65], start=True,
                                         stop=True)
                        nc.scalar.copy(ad_sb[:, t:t + 1], ps2[:, :])
                base = s * SUBROWS
                for sci in range(nsuper_sub):
                    c0 = sci * SCH
                    nch = min(SCH, nchunk_sub - c0)
                    cols = 128 * nch
                    xt = xin.tile([kdim, 1024], bf16, tag="xt")
                    nc.sync.dma_start(
                        out=xt[:, 0:cols],
                        in_=hT[:, base + 128 * c0:base + 128 * c0 + cols])
                    ps = psA.tile([128, 512], f32, tag="ps")
                    for j in range(nch):
                        nc.tensor.matmul(ps[:, 64 * j:64 * (j + 1)],
                                         xt[:, 128 * j:128 * (j + 1)],
                                         wse_sb[:, 0:64], start=True, stop=True)
                    st = stage.tile([128, 512], f32, tag="st")
                    nc.vector.tensor_copy(st[:, 0:64 * nch], ps[:, 0:64 * nch])
                    dst = tabl[base + 128 * c0:base + 128 * c0 + cols, :] \
                        .rearrange("(c p) f -> p c f", p=128)
                    nc.scalar.dma_start(out=dst, in_=st[:, 0:64 * nch]
                                        .rearrange("p (c f) -> p c f", f=64))
                # pad row of this sub-table
                nc.sync.dma_start(out=tabl[base:base + 1, :], in_=padrow[0:1, :])

            # Stage B: gather groups. Desc-gen (prepare_only) has no table
            # dependency — it runs from t=0, overlapped with Stage A. All of
            # a sub-shard's preps are emitted before its ONE trigger so the
            # trigger's table-read wait never stalls later desc-gen.
            def compute_group(tg, dma_sem, tiles):
                ng = len(tiles)
                # tg consumers are all Vector ops; the prep's tick only covers
                # desc-gen, so gate Vector on the DMA-completion sem itself.
                nc.vector.wait_ge(dma_sem, 16)
                den_g = sm.tile([128, ng], f32, tag="den")
                p_list = []
                for i, (t, toff, k) in enumerate(tiles):
                    as_t = sm.tile([128, k], f32, tag=f"as{i}")
                    nc.vector.tensor_reduce(as_t[:, :],
                                            tg[:, toff:toff + k, :],
                                            mybir.AxisListType.X,
                                            mybir.AluOpType.add)
                    z_t = sm.tile([128, k], f32, tag=f"z{i}")
                    nc.vector.tensor_tensor(z_t[:, :], as_t[:, :],
                                            ad_sb[:, t:t + 1]
                                            .broadcast_to([128, k]),
                                            mybir.AluOpType.add)
                    s_t = sm.tile([128, k], f32, tag=f"s{i}")
                    nc.vector.scalar_tensor_tensor(s_t[:, :], z_t[:, :],
                                                   NEG_SLOPE, z_t[:, :],
                                                   mybir.AluOpType.mult,
                                                   mybir.AluOpType.max)
                    p_t = sm.tile([128, k], f32, tag=f"p{i}")
                    nc.scalar.activation(p_t[:, :], s_t[:, :],
                                         mybir.ActivationFunctionType.Exp,
                                         accum_out=den_g[:, i:i + 1])
                    p_list.append(p_t)
                rd_g = sm.tile([128, ng], f32, tag="rd")
                nc.vector.reciprocal(rd_g[:, :], den_g[:, :])
                for i, (t, toff, k) in enumerate(tiles):
                    p_t = p_list[i]
                    pt = ptp.tile([128, k, 64], f32, tag="pt")
                    p_b = p_t[:, :].unsqueeze(2).broadcast_to([128, k, 64])
                    nc.vector.tensor_tensor(pt[:, 0:k, :],
                                            tg[:, toff:toff + k, :], p_b,
                                            mybir.AluOpType.mult)
                    u = sm.tile([128, 64], f32, tag=f"u{i}")
                    nc.vector.tensor_reduce(u[:, :],
                                            pt[:, 0:k, :].transpose([0, 2, 1]),
                                            mybir.AxisListType.X,
                                            mybir.AluOpType.add)
                    o1 = sm.tile([128, 64], f32, tag=f"o1{i}")
                    nc.vector.scalar_tensor_tensor(o1[:, :], u[:, :],
                                                   rd_g[:, i:i + 1],
                                                   rb_sb[:, 0:64],
                                                   mybir.AluOpType.mult,
                                                   mybir.AluOpType.mult)
                    o2 = sm.tile([128, 64], f32, tag=f"o2{i}")
                    nc.vector.tensor_tensor(o2[:, :], o1[:, :],
                                            rb_sb[:, 64:128],
                                            mybir.AluOpType.add)
                    nc.sync.dma_start(out=outp[128 * t:128 * (t + 1), :],
                                      in_=o2[:, :])

            for gi, (s, goff, KG, tiles) in enumerate(ginfo):
                base = s * SUBROWS
                tg = tgp.tile([128, KG, 64], f32, tag="tg")
                dma_sem = nc.alloc_semaphore(f"swdge_g{gi}")
                nc.gpsimd.dma_gather(tg[:, :, :],
                                     tabl[base:base + SUBROWS, :],
                                     idx_sb[:, goff:goff + 8 * KG],
                                     128 * KG, 128 * KG, 64,
                                     single_packet=False,
                                     prepare_only=True, sem=dma_sem)
                nc.gpsimd.trigger_dma(count=None)
                compute_group(tg, dma_sem, tiles)

    nc.compile()
    return nc


LAST = {}


def _assemble_hT(featT_bf, loc, n_cs, SUBROWS, kdim):
    """Per-core hT [kdim, NSUB*SUBROWS] bf16 with sub-table column layout."""
    hts = []
    for c in range(C):
        ht = np.zeros((kdim, NSUB * SUBROWS), featT_bf.dtype)
        for s in range(NSUB):
            n = int(n_cs[c, s])
            nodes = np.nonzero(loc[c, s])[0]
            # loc values are 1..n in node order (np.unique sorted)
            ht[:, s * SUBROWS + 1:s * SUBROWS + 1 + n] = featT_bf[:, nodes]
        hts.append(ht)
    return hts


def kernel(x, edge_index, W1, a_src1, a_dst1, b1, W2, a_src2, a_dst2, b2):
    from concourse.bass_utils import run_bass_kernel_spmd
    import ml_dtypes

    bf = np.dtype(ml_dtypes.bfloat16)

    x = np.asarray(x, np.float32)
    edge_index = np.asarray(edge_index)
    W1 = np.asarray(W1, np.float32); a_src1 = np.asarray(a_src1, np.float32)
    a_dst1 = np.asarray(a_dst1, np.float32); b1 = np.asarray(b1, np.float32)
    W2 = np.asarray(W2, np.float32); a_src2 = np.asarray(a_src2, np.float32)
    a_dst2 = np.asarray(a_dst2, np.float32); b2 = np.asarray(b2, np.float32)

    orders, pos_of, plan, groups, loc, n_cs, SUBROWS = _build_plan(edge_index)
    idx, ginfo, idxcols = _build_idx_tensor(plan, groups, loc)

    nc1 = _build_launch(IN, ginfo, idxcols, SUBROWS)
    nc2 = _build_launch(OUT, ginfo, idxcols, SUBROWS)

    def guard(a):
        return np.where(a == 0, np.float32(1e-30), a)

    w1se = np.concatenate([W1 * a_src1[None, :], (W1 @ a_dst1)[:, None]], 1).astype(bf)
    w2se = np.concatenate([W2 * a_src2[None, :], (W2 @ a_dst2)[:, None]], 1).astype(bf)
    rb1 = np.concatenate([np.tile(1.0 / guard(a_src1), (128, 1)),
                          np.tile(b1, (128, 1))], 1).astype(np.float32)
    rb2 = np.concatenate([np.tile(1.0 / guard(a_src2), (128, 1)),
                          np.tile(b2, (128, 1))], 1).astype(np.float32)

    # layer 1 inputs
    xT_bf = np.ascontiguousarray(x.T).astype(bf)            # [IN, N]
    hts1 = _assemble_hT(xT_bf, loc, n_cs, SUBROWS, IN)
    in_maps1 = []
    for c in range(C):
        own = orders[c]
        hoT = np.zeros((IN, NSHP), bf)
        real = own >= 0
        hoT[:, real] = xT_bf[:, own[real]]
        in_maps1.append({"hT": hts1[c], "hoT": hoT, "wse": w1se, "rb": rb1,
                        "idx": idx[c]})

    res1 = run_bass_kernel_spmd(nc1, in_maps1, core_ids=list(range(C)))
    LAST["res1"] = res1

    # h2 per node from pi-order shards
    h2 = np.zeros((N, OUT), np.float32)
    for c in range(C):
        sh = np.asarray(res1.results[c]["outp"])
        own = orders[c]
        real = own >= 0
        h2[own[real]] = sh[real]
    h2T_bf = np.ascontiguousarray(h2.T).astype(bf)          # [64, N]

    hts2 = _assemble_hT(h2T_bf, loc, n_cs, SUBROWS, OUT)
    in_maps2 = []
    for c in range(C):
        own = orders[c]
        hoT2 = np.zeros((OUT, NSHP), bf)
        real = own >= 0
        hoT2[:, real] = h2T_bf[:, own[real]]
        in_maps2.append({"hT": hts2[c], "hoT": hoT2, "wse": w2se, "rb": rb2,
                        "idx": idx[c]})

    res2 = run_bass_kernel_spmd(nc2, in_maps2, core_ids=list(range(C)))
    LAST["res2"] = res2

    out = np.empty((N, OUT), np.float32)
    for c in range(C):
        sh = np.asarray(res2.results[c]["outp"])
        own = orders[c]
        real = own >= 0
        out[own[real]] = sh[real]
    return out
